# revision 15
# baseline (speedup 1.0000x reference)
"""Trainium2 Bass kernel for causal multi-head attention (B=2, S=2048, D=1024, 16 heads x 64).

Sharding: 8 cores = 2 batches x 4 head-groups (tensor parallel over heads),
collective-free. Each core computes attention for its 4 heads over the full
sequence AND applies its own 4-head slice of W_O to all q positions,
producing a transposed partial output [D, S] that the host transposes and
sums across the 4 cores of each batch (the "all-reduce" moves to the host,
off the device critical path).

Attention is flash-style with transposed scores: sT[k, q] = K Q^T (keys on
partitions). Both heads of a pair write one [128, 1024] 2-bank PSUM tile so a
single ACT exp covers them. AV uses stationary [v | 1] so PSUM row 64
accumulates the softmax denominator for free; the denominator row is
broadcast to 64 partitions by a matmul and reciprocated at base partition 0
with the fast custom DVE op. The normalized outputs of a head pair are
stacked on partitions 0-63 / 64-127 of a u2 tile (the odd head hops through
a small SBUF->SBUF DMA since DVE cannot cross partitions), which makes the
out-projection contraction a clean pair of full-128-deep matmuls per output
block. QKV projection and out-projection chains are chopped into
single-instruction units and interleaved between attention rounds as PE
filler.
"""

import os
import sys

sys.path.insert(0, "/opt/trn_rl_repo")

import numpy as np

# ---- problem constants (hardcoded; kernel.py must be self-contained) ----
B = 2
S = 2048
D = 1024
N_HEADS = 16
DH = 64                 # head dim
NCORES = 8
NH_CORE = N_HEADS // 4  # 4 heads per core (4-way TP x 2-way batch DP)
SCALE = 1.0 / 8.0       # 1/sqrt(64)

P = 128                 # partitions
DC = D // P             # 8 contraction chunks for the projections
KC = S // P             # 16 key chunks
QT = 512                # q tile width (free dim) per quarter
NQT = S // QT           # 4 q tiles
GRP = 4                 # cores per batch group

_CACHE = {}


def _build():
    import concourse.bass as bass
    import concourse.tile as tile
    from concourse import bacc, mybir

    f32 = mybir.dt.float32
    F16 = mybir.dt.float16

    nc = bacc.Bacc(
        "TRN2",
        target_bir_lowering=False,
        debug=False,
        enable_asserts=False,
        num_devices=NCORES,
    )

    # all inputs pre-chunked host-side to [128, DC, n] so DMAs are contiguous
    xt_d = nc.dram_tensor("xt", [P, DC, S], F16, kind="ExternalInput").ap()
    wqt_d = nc.dram_tensor("wqt", [P, DC, NH_CORE * DH], F16, kind="ExternalInput").ap()
    wkt_d = nc.dram_tensor("wkt", [P, DC, NH_CORE * DH], F16, kind="ExternalInput").ap()
    wvt_d = nc.dram_tensor("wvt", [P, DC, NH_CORE * DH], F16, kind="ExternalInput").ap()
    # own-head W_O, pair-stacked: [128 partitions = (even head hd | odd head
    # hd), pair, D]
    wop_d = nc.dram_tensor("wop", [P, 2, D], F16, kind="ExternalInput").ap()
    msk_d = nc.dram_tensor("msk", [P, P], F16, kind="ExternalInput").ap()
    # transposed partial output: [qt, dblock, 128 d, 512 q]
    out_d = nc.dram_tensor("out", [NQT * DC * P, QT], F16, kind="ExternalOutput").ap()

    Exp = mybir.ActivationFunctionType.Exp

    with tile.TileContext(nc) as tc:
        with (
            tc.tile_pool(name="const", bufs=1) as const,
            tc.tile_pool(name="work", bufs=2) as work,
            tc.tile_pool(name="ps", bufs=1, space="PSUM") as ps_pool,
        ):
            # ---------------- input DMAs ----------------
            # Only the sync/scalar HWDGE rings move real bandwidth (gpsimd
            # dma_start is the slow SWDGE Q7 path); the two rings stream in
            # parallel and share the ~358 GB/s HBM port. Order by first use:
            # the q/k chains gate on (wq|wk) + xt quarter 0, the v chains on
            # wv, everything else trails.
            wq_sb = const.tile([P, DC, NH_CORE * DH], F16)
            nc.sync.dma_start(wq_sb[:], wqt_d)
            xt_sb = const.tile([P, DC, S], F16)
            nc.scalar.dma_start(xt_sb[:, 0:4, 0:QT], xt_d[:, 0:4, 0:QT])
            nc.scalar.dma_start(xt_sb[:, 4:DC, 0:QT], xt_d[:, 4:DC, 0:QT])
            wk_sb = const.tile([P, DC, NH_CORE * DH], F16)
            nc.sync.dma_start(wk_sb[:], wkt_d)
            wv_sb = const.tile([P, DC, NH_CORE * DH], F16)
            nc.sync.dma_start(wv_sb[:], wvt_d)
            tri_sb = const.tile([P, P], F16)
            nc.sync.dma_start(tri_sb[:], msk_d)
            half = (S - QT) // 2
            nc.scalar.dma_start(
                xt_sb[:, :, QT : QT + half], xt_d[:, :, QT : QT + half]
            )
            nc.sync.dma_start(
                xt_sb[:, :, QT + half : S], xt_d[:, :, QT + half : S]
            )
            wo_sb = const.tile([P, 2, D], F16)
            nc.scalar.dma_start(wo_sb[:], wop_d)

            # ---------------- SBUF state ----------------
            qT = [const.tile([P, S], F16, name=f"qT{i}") for i in range(2)]
            kT = [const.tile([P, S], F16, name=f"kT{i}") for i in range(2)]
            v_aug = [const.tile([P, KC, DH + 1], F16, name=f"vaug{h}") for h in range(NH_CORE)]
            ones_f32 = const.tile([P, DH], f32)
            nc.vector.memset(ones_f32[:], 1.0)
            ones_f16 = const.tile([DH + 1, DH], F16)
            nc.vector.memset(ones_f16[:], 1.0)
            for h in range(NH_CORE):
                nc.gpsimd.tensor_copy(v_aug[h][:, :, DH : DH + 1], ones_f32[:, 0:KC, None])

            # per-quarter normalized pair-stacked attention outputs
            u2 = {}

            pj = [0]  # alternating tag counter for the 2 shared psum banks

            def _pj_tile(shape, name):
                t = ps_pool.tile(shape, f32, name=name, tag=f"pj{pj[0] % 2}", bufs=1)
                pj[0] += 1
                return t

            # ---- filler units: single instructions emitted between rounds ----
            def qk_chain_units(nt, pr, w_sb, dst):
                st = {}
                us = []
                for dc in range(DC):
                    def mm(dc=dc, nt=nt, pr=pr, w_sb=w_sb):
                        if dc == 0:
                            st["pp"] = _pj_tile([P, QT], "pp")
                        nc.tensor.matmul(
                            st["pp"][:],
                            w_sb[:, dc, pr * P : (pr + 1) * P],
                            xt_sb[:, dc, nt * QT : (nt + 1) * QT],
                            start=(dc == 0),
                            stop=(dc == DC - 1),
                        )
                    us.append(mm)
                def cp(nt=nt, dst=dst):
                    nc.vector.tensor_copy(dst[:, nt * QT : (nt + 1) * QT], st["pp"][:])
                us.append(cp)
                return us

            def v_chain_units(pc):
                st = {}
                us = []
                for dc in range(DC):
                    def mm(dc=dc, pc=pc):
                        if dc == 0:
                            st["vp"] = _pj_tile([P, NH_CORE * DH], "vp")
                        nc.tensor.matmul(
                            st["vp"][:],
                            xt_sb[:, dc, pc * P : (pc + 1) * P],
                            wv_sb[:, dc, :],
                            start=(dc == 0),
                            stop=(dc == DC - 1),
                        )
                    us.append(mm)
                for h in range(NH_CORE):
                    def cp(h=h, pc=pc):
                        nc.vector.tensor_copy(
                            v_aug[h][:, pc, 0:DH], st["vp"][:, h * DH : (h + 1) * DH]
                        )
                    us.append(cp)
                return us

            def proj_units(nt, prs=(0, 1), with_v=True):
                us = []
                for pr in prs:
                    us += qk_chain_units(nt, pr, wq_sb, qT[pr])
                    us += qk_chain_units(nt, pr, wk_sb, kT[pr])
                if with_v:
                    for pc in range(4 * nt, 4 * nt + 4):
                        us += v_chain_units(pc)
                return us

            def out_proj_units(qt):
                """Own-head out-projection for quarter qt: per d-block, two
                full-128-deep accumulating matmuls (one per head pair) over
                the pair-stacked u2(qt), then evacuate + store transposed."""
                u2q = u2[qt]
                st = {}
                us = []
                for db in range(DC):
                    def mm0(db=db):
                        st["op"] = _pj_tile([P, QT], "op")
                        nc.tensor.matmul(
                            st["op"][:],
                            wo_sb[:, 0, db * P : (db + 1) * P],
                            u2q[:, 0, :],
                            start=True,
                            stop=False,
                        )
                    def mm1(db=db):
                        nc.tensor.matmul(
                            st["op"][:],
                            wo_sb[:, 1, db * P : (db + 1) * P],
                            u2q[:, 1, :],
                            start=False,
                            stop=True,
                        )
                    def cp(db=db, qt=qt):
                        osb = work.tile([P, QT], F16, name="osb", bufs=3)
                        st["osb"] = osb
                        nc.vector.tensor_copy(osb[:], st["op"][:])
                    def dm(db=db, qt=qt):
                        row = (qt * DC + db) * P
                        nc.sync.dma_start(out_d[row : row + P, :], st["osb"][:])
                    us += [mm0, mm1, cp, dm]
                return us

            def out_proj_a_units(qt):
                """Tail-shortening pass A for the last quarter: project pair 0
                (complete after pr=0's norm) into f16 partials while pr=1's
                attention still runs."""
                u2q = u2[qt]
                st = {}
                parts = []
                us = []
                for db in range(DC):
                    def mma(db=db):
                        st["op"] = _pj_tile([P, QT], "op")
                        nc.tensor.matmul(
                            st["op"][:],
                            wo_sb[:, 0, db * P : (db + 1) * P],
                            u2q[:, 0, :],
                            start=True,
                            stop=True,
                        )
                    def cpa(db=db):
                        part = work.tile([P, QT], F16, name="opart", bufs=DC)
                        parts.append(part)
                        nc.vector.tensor_copy(part[:], st["op"][:])
                    us += [mma, cpa]
                return us, parts

            def out_proj_b_units(qt, parts):
                """Pass B: pair 1 matmul + add pass-A partial, evacuate, store."""
                u2q = u2[qt]
                st = {}
                us = []
                for db in range(DC):
                    def mmb(db=db):
                        st["op"] = _pj_tile([P, QT], "op")
                        nc.tensor.matmul(
                            st["op"][:],
                            wo_sb[:, 1, db * P : (db + 1) * P],
                            u2q[:, 1, :],
                            start=True,
                            stop=True,
                        )
                    def cpb(db=db):
                        osb = work.tile([P, QT], F16, name="osb", bufs=3)
                        st["osb"] = osb
                        nc.vector.tensor_add(osb[:], st["op"][:], parts[db][:])
                    def dmb(db=db, qt=qt):
                        row = (qt * DC + db) * P
                        nc.sync.dma_start(out_d[row : row + P, :], st["osb"][:])
                    us += [mmb, cpb, dmb]
                return us

            units = []

            def fill(rounds_left):
                if not units:
                    return
                n = max(1, (len(units) + rounds_left - 1) // max(rounds_left, 1))
                for _ in range(min(n, len(units))):
                    units.pop(0)()

            def flush():
                while units:
                    units.pop(0)()

            def norm_store(qt, pr, dens, accs):
                """Normalize the head pair and stack into u2[qt][:, pr, :]:
                even head straight from PSUM at lanes 0-63, odd head via a
                small SBUF->SBUF DMA hop to lanes 64-127."""
                u2q = u2[qt]
                for h2 in range(2):
                    rb_ps = _pj_tile([DH, QT], "rb")
                    nc.tensor.matmul(
                        rb_ps[:],
                        ones_f16[DH : DH + 1, :],
                        dens[h2][DH : DH + 1, :],
                        start=True,
                        stop=True,
                    )
                    rb_sb = work.tile([DH, QT], f32, name="rb_sb", bufs=4)
                    nc.vector.reciprocal_approx_fast(rb_sb[:], rb_ps[:])
                    if h2 == 0:
                        nc.vector.tensor_mul(
                            u2q[0:DH, pr, :], accs[h2][0:DH, :], rb_sb[:]
                        )
                    else:
                        tmp = work.tile([DH, QT], F16, name="u_tmp", bufs=2)
                        nc.vector.tensor_mul(tmp[:], accs[h2][0:DH, :], rb_sb[:])
                        # scalar queue: idle post-exp, and keeps the hop off
                        # the gpsimd queue whose end-of-pool drain would gate
                        # the final out-projection
                        nc.scalar.dma_start(u2q[DH:P, pr, :], tmp[:])

            # ---------------- main loop ----------------
            # upfront: only what gates qt0 pr0's first scores + first AV; the
            # remaining v chains and pr1's q/k chains become the first fillers
            for u in proj_units(0, prs=(0,), with_v=False):
                u()
            for u in v_chain_units(0):
                u()
            for qt in range(NQT):
                u2[qt] = work.tile([P, 2, QT], F16, name="u2", bufs=2)
                # filler work for this quarter's ACT-bound attention span
                if qt == 0:
                    for pc in range(1, 4):
                        units.extend(v_chain_units(pc))
                    units.extend(proj_units(0, prs=(1,), with_v=False))
                if qt + 1 < NQT:
                    units.extend(proj_units(qt + 1))
                if qt >= 1:
                    units.extend(out_proj_units(qt - 1))

                nk = (qt + 1) * (QT // P)
                rounds_left = 2 * nk
                for pr in range(2):
                    acc = [
                        ps_pool.tile(
                            [DH + 1, QT], f32, name=f"acc{h2}", tag=f"acc{h2}", bufs=1
                        )
                        for h2 in range(2)
                    ]

                    def av_round(kb, r, pat):
                        for h2 in range(2):
                            nc.tensor.matmul(
                                acc[h2][0 : DH + 1, r:QT],
                                v_aug[pr * 2 + h2][:, kb, :],
                                pat[:, h2 * QT + r : (h2 + 1) * QT],
                                start=(kb == 0),
                                stop=(kb == nk - 1),
                            )

                    pend = None  # (kb, r, pat) awaiting its AV matmuls
                    for kb in range(nk):
                        k_sl = slice(kb * P, (kb + 1) * P)
                        ri = kb - qt * (QT // P)  # >= 0 on diagonal tiles
                        r = max(ri, 0) * P        # first valid col in this q tile
                        c_sl = slice(qt * QT + r, (qt + 1) * QT)
                        sc = ps_pool.tile([P, 2 * QT], f32, name="sc", tag="sc", bufs=2)
                        pat = work.tile([P, 2 * QT], F16, name="pat", bufs=4)
                        for h2 in range(2):
                            hb = h2 * DH
                            # explicit row-group placement: the two 64-row
                            # stationaries occupy disjoint halves of the PE
                            # array so their LDWEIGHTS+MATMULs can overlap
                            nc.tensor.matmul(
                                sc[:, h2 * QT + r : (h2 + 1) * QT],
                                kT[pr][hb : hb + DH, k_sl],
                                qT[pr][hb : hb + DH, c_sl],
                                start=True,
                                stop=True,
                                tile_position=(hb, 0),
                            )
                        # one exp covers both heads (cols 512..512+r of the
                        # diagonal rounds are stale-PSUM garbage, never read)
                        nc.scalar.activation(
                            pat[:, r : 2 * QT], sc[:, r : 2 * QT], Exp, scale=SCALE
                        )
                        if ri >= 0:
                            for h2 in range(2):
                                nc.vector.tensor_mul(
                                    pat[:, h2 * QT + r : h2 * QT + r + P],
                                    pat[:, h2 * QT + r : h2 * QT + r + P],
                                    tri_sb[:],
                                )
                        # software-pipeline the PE stream one round deep
                        if pend is not None:
                            av_round(*pend)
                        pend = (kb, r, pat)
                        rounds_left -= 1
                        fill(rounds_left)
                    if pend is not None:
                        av_round(*pend)
                    dens = []
                    for h2 in range(2):
                        # f16 denominator stays on lane 64 (DVE can't move
                        # across partitions); the matmul streams from there
                        den = work.tile([DH + 1, QT], F16, name="den", bufs=6)
                        nc.vector.tensor_copy(
                            den[DH : DH + 1, :], acc[h2][DH : DH + 1, :]
                        )
                        dens.append(den)
                    norm_store(qt, pr, dens, acc)
                    if qt == NQT - 1 and pr == 0:
                        a_us, op3_parts = out_proj_a_units(NQT - 1)
                        units.extend(a_us)
            flush()
            for u in out_proj_b_units(NQT - 1, op3_parts):
                u()

    nc.compile()
    return nc


def _get_nc():
    if "nc" not in _CACHE:
        _CACHE["nc"] = _build()
    return _CACHE["nc"]


def _tri():
    k = np.arange(P)[:, None]
    q = np.arange(P)[None, :]
    return (q >= k).astype(np.float32)


def _ensure_ntff_hook():
    """Register the axon NTFF profile hook (missing antenv.axon_hooks shim)."""
    import types

    try:
        from antenv.axon_hooks import get_axon_ntff_profile_hook  # noqa: F401

        return
    except ImportError:
        pass
    import antenv

    if "/root/.axon_site" not in sys.path:
        sys.path.insert(0, "/root/.axon_site")
    from trn_agent_boot.trn_boot import _ntff_profile_via_ctypes

    hook = _ntff_profile_via_ctypes("/opt/axon/libaxon_pjrt.so")
    mod = types.ModuleType("antenv.axon_hooks")
    mod.get_axon_ntff_profile_hook = lambda: hook
    mod.set_axon_ntff_profile_hook = lambda h: None
    sys.modules["antenv.axon_hooks"] = mod
    antenv.axon_hooks = mod


def kernel(residual, W_Q, W_K, W_V, W_O):
    from concourse.bass_utils import run_bass_kernel_spmd

    if int(os.environ.get("KERNEL_TRACE", "0")):
        _ensure_ntff_hook()

    residual = np.ascontiguousarray(np.asarray(residual), np.float32)
    W_Q = np.ascontiguousarray(np.asarray(W_Q), np.float32)
    W_K = np.ascontiguousarray(np.asarray(W_K), np.float32)
    W_V = np.ascontiguousarray(np.asarray(W_V), np.float32)
    W_O = np.ascontiguousarray(np.asarray(W_O), np.float32)

    nc = _get_nc()
    tri = _tri()

    def chunked(a):
        # [D, n] -> [128, DC, n] so every DMA row is contiguous
        n = a.shape[1]
        return np.ascontiguousarray(
            a.reshape(DC, P, n).transpose(1, 0, 2).astype(np.float16)
        )

    in_maps = []
    for c in range(NCORES):
        b, g = divmod(c, GRP)
        hs = slice(g * NH_CORE, (g + 1) * NH_CORE)
        # pair-stack own-head W_O: [pair, odd, hd, D] -> [odd*64+hd, pair, D]
        wop = (
            W_O[hs]
            .reshape(2, 2, DH, D)
            .transpose(1, 2, 0, 3)
            .reshape(P, 2, D)
            .astype(np.float16)
        )
        in_maps.append(
            {
                "xt": chunked(residual[b].T),
                "wqt": chunked(W_Q[hs].transpose(2, 0, 1).reshape(D, NH_CORE * DH)),
                "wkt": chunked(W_K[hs].transpose(2, 0, 1).reshape(D, NH_CORE * DH)),
                "wvt": chunked(W_V[hs].transpose(2, 0, 1).reshape(D, NH_CORE * DH)),
                "wop": np.ascontiguousarray(wop),
                "msk": tri.astype(np.float16),
            }
        )

    res = run_bass_kernel_spmd(
        nc,
        in_maps,
        core_ids=list(range(NCORES)),
        trace=bool(int(os.environ.get("KERNEL_TRACE", "0"))),
        trace_cores=(
            list(range(NCORES))
            if int(os.environ.get("KERNEL_TRACE_ALL", "0"))
            else [0] if int(os.environ.get("KERNEL_TRACE", "0")) else None
        ),
    )
    _CACHE["last_results"] = res

    # host-side unshard: transpose each core's partial [qt, db, d, q] ->
    # [s, d] and sum the 4 head-group partials of each batch
    out = np.zeros((B, S, D), np.float32)
    for c in range(NCORES):
        b = c // GRP
        blk = np.asarray(res.results[c]["out"], np.float32).reshape(NQT, DC, P, QT)
        out[b] += blk.transpose(0, 3, 1, 2).reshape(S, D)
    return out
